# revision 6
# baseline (speedup 1.0000x reference)
"""GNN message-passing layer on 8 TRN2 NeuronCores.

Computes out = relu((adj^T @ x / deg) @ U^T) for N=8192 nodes, D=512 dims.

Sharding: columns of adj (= output rows) are split across the 8 cores;
x and U are replicated, so each core computes a [1024, 512] output slab
with no collectives.

Host-side restaging (pure layout shuffles, no arithmetic): every DRAM
tensor is laid out partition-major so each SBUF partition reads one long
contiguous run; the int32->bf16 and f32->bf16 casts ride the DMA engines.

Single-pass structure: the per-core DMA bus (~360-385 GB/s) is the
binding roofline (53.3 MB of traffic), so both 512-column output halves
accumulate simultaneously in all 8 PSUM banks while adj (both halves)
and x stream together through one j-loop - the DMA queues stay
saturated for the whole kernel instead of front-loading x (which made
the old second half PE-bound with idle DMA).

deg rides the stream as bf16 per-partition partials on the DVE, then
ones-weight matmuls + PE transposes at the tail (exact: counts <= 256).
The final group is issued chunk-major with evacuation, deg, and the
first half's output matmuls interleaved so the post-stream tail is only
the last evac + 16 output matmuls + relu + store.
"""

import sys

if "/opt/trn_rl_repo" not in sys.path:
    sys.path.insert(0, "/opt/trn_rl_repo")

import numpy as np

import concourse.bacc as bacc
import concourse.mybir as mybir
import concourse.tile as tile
from concourse.bass_utils import run_bass_kernel_spmd

N = 8192          # nodes
D = 512           # node dim
NCORES = 8
SH = N // NCORES  # 1024 adj columns (output rows) per core
NJ = N // 128     # 64 contraction tiles
XG = 8            # j-tiles per load group
NG = NJ // XG     # 8 groups
F32 = mybir.dt.float32
BF16 = mybir.dt.bfloat16
I32 = mybir.dt.int32

_compiled = None


def _build():
    nc = bacc.Bacc("TRN2", target_bir_lowering=False, debug=False, num_devices=NCORES)
    # partition-major layouts (see _run for the host-side shuffles)
    x_d = nc.dram_tensor("x", [128, NJ, D], F32, kind="ExternalInput").ap()
    adj_d = nc.dram_tensor("adj", [2, 128, NJ, D], I32, kind="ExternalInput").ap()
    ut_d = nc.dram_tensor("ut", [128, 4, D], F32, kind="ExternalInput").ap()
    out_d = nc.dram_tensor("out", [128, 8, D], F32, kind="ExternalOutput").ap()

    with tile.TileContext(nc) as tc:
        with (
            tc.tile_pool(name="xw", bufs=3) as xw_pool,
            tc.tile_pool(name="xs", bufs=2) as xs_pool,
            tc.tile_pool(name="abf", bufs=6) as abf_pool,
            tc.tile_pool(name="cons", bufs=1) as cons_pool,
            tc.tile_pool(name="degp", bufs=1) as degp_pool,
            tc.tile_pool(name="evac", bufs=1) as evac_pool,
            tc.tile_pool(name="osb", bufs=2) as osb_pool,
            tc.tile_pool(name="pacc", bufs=1, space="PSUM") as pacc_pool,
        ):
            ones = cons_pool.tile([128, 128], BF16)
            nc.vector.memset(ones[:], 1.0)
            # f32 identity for PE-transpose of the deg row
            ident = cons_pool.tile([128, 128], F32)
            nc.vector.memset(ident[:], 1.0)
            nc.gpsimd.affine_select(
                ident[:], ident[:], pattern=[[-1, 128]], base=0,
                channel_multiplier=1,
                compare_op=mybir.AluOpType.is_equal, fill=0.0,
            )
            u_bf = cons_pool.tile([128, 4, D], BF16)
            nc.gpsimd.dma_start(u_bf[:], ut_d[:])

            # 8 PSUM accumulators: [h][c] chunk of aggT
            agg_ps = [
                [
                    pacc_pool.tile([128, D], F32, tag=f"agg{h}{c}", name=f"agg{h}{c}")
                    for c in range(4)
                ]
                for h in range(2)
            ]
            agg_sc = [
                [
                    evac_pool.tile([128, D], BF16, tag=f"aggsc{h}{c}",
                                   name=f"aggsc{h}{c}")
                    for c in range(4)
                ]
                for h in range(2)
            ]
            # per-partition partial degree counts; lane values stay <= NG so
            # bf16 accumulation is exact
            degp = [
                degp_pool.tile([128, XG, D], BF16, tag=f"degp{h}", name=f"degp{h}")
                for h in range(2)
            ]
            for h in range(2):
                nc.vector.memset(degp[h][:], 0.0)

            # startup PE filler: ramp the clock / pstate while the first
            # group's DMA lands; garbage lands in agg banks and is reset by
            # the j==0 start=True matmuls
            for f in range(12):
                nc.tensor.matmul(
                    agg_ps[f % 2][(f // 2) % 4][:, 0:128], ones[:], ones[:],
                    start=True, stop=True, skip_group_check=True,
                )

            xg_tiles = [None] * NG
            a_tiles = [[None] * NG for _ in range(2)]

            def load_group(g):
                # x and adj both cast to bf16 inline in the DMA engines;
                # x+out ride the sync HWDGE queue, adj the gpsimd SWDGE
                # queue (packet-level round-robin in the 16 SDMA engines)
                xs = xs_pool.tile([128, XG, D], F32, tag="xs", name=f"xs{g}")
                nc.sync.dma_start(xs[:], x_d[:, g * XG:(g + 1) * XG, :])
                xg = xw_pool.tile([128, XG, D], BF16, tag="xg", name=f"xg{g}")
                nc.scalar.copy(xg[:], xs[:])
                xg_tiles[g] = xg
                for h in range(2):
                    a_bf = abf_pool.tile([128, XG, D], BF16, tag="abf",
                                         name=f"abf{h}_{g}")
                    nc.gpsimd.dma_start(
                        a_bf[:], adj_d[h, :, g * XG:(g + 1) * XG, :]
                    )
                    a_tiles[h][g] = a_bf

            load_group(0)
            load_group(1)

            def mm_tile(h, g, t, c):
                j = g * XG + t
                nc.tensor.matmul(
                    agg_ps[h][c][:],
                    xg_tiles[g][:, t, c * 128:(c + 1) * 128],
                    a_tiles[h][g][:, t, :],
                    start=j == 0,
                    stop=j == NJ - 1,
                )

            # ---- steady-state j-loop over groups 0..NG-2 ----
            for g in range(NG - 1):
                if g + 2 < NG:
                    load_group(g + 2)
                for h in range(2):
                    nc.vector.tensor_add(degp[h][:], degp[h][:], a_tiles[h][g][:])
                for t in range(XG):
                    for c in range(4):
                        mm_tile(0, g, t, c)
                        mm_tile(1, g, t, c)

            # ---- final group: chunk-major so each accumulator closes as
            # early as possible; evac/deg/out work interleaves with the
            # remaining chunks' matmuls ----
            g = NG - 1
            for h in range(2):
                nc.vector.tensor_add(degp[h][:], degp[h][:], a_tiles[h][g][:])

            deg_sb = [None, None]
            recipt = [None, None]

            def deg_pipeline(h, bank_mm, bank_tr):
                # partition+lane sum of degp via accumulating ones-matmuls
                # into a freed agg bank, then PE-transpose into
                # per-partition layout for the output scale
                deg_ps = pacc_pool.tile([128, D], F32, tag=bank_mm, name=f"degps{h}")
                for t in range(XG):
                    nc.tensor.matmul(
                        deg_ps[:], ones[:], degp[h][:, t, :],
                        start=t == 0, stop=t == XG - 1,
                    )
                dsb = evac_pool.tile([128, D], F32, tag=f"degsb{h}", name=f"degsb{h}")
                nc.scalar.copy(dsb[:], deg_ps[:])
                deg_sb[h] = dsb
                degt_ps = pacc_pool.tile([128, 4, 128], F32, tag=bank_tr,
                                         name=f"degt{h}")
                for ic in range(4):
                    nc.tensor.transpose(
                        degt_ps[:, ic, :],
                        dsb[:, ic * 128:(ic + 1) * 128],
                        ident[:],
                    )
                rt = evac_pool.tile([128, 4], F32, tag=f"recipt{h}", name=f"recipt{h}")
                nc.vector.reciprocal_approx_fast(rt[:], degt_ps[:, :, 0])
                recipt[h] = rt

            # order: h1 chunks first, then h0, so that h1's evacuations and
            # output matmuls hide inside h0's final accumulation, and only
            # h0's tail remains after the stream ends
            def close_chunk(h, c, evac_engine):
                for t in range(XG):
                    mm_tile(h, g, t, c)
                # evacuate on ACT or DVE so the two alternate
                if evac_engine == "act":
                    nc.scalar.copy(agg_sc[h][c][:], agg_ps[h][c][:])
                else:
                    nc.vector.tensor_copy(agg_sc[h][c][:], agg_ps[h][c][:])

            def out_pipeline(h, bank0, bank1):
                for ic in range(4):
                    out_ps = pacc_pool.tile([128, D], F32,
                                            tag=(bank0 if ic % 2 == 0 else bank1),
                                            name=f"outps{h}{ic}")
                    for c in range(4):
                        nc.tensor.matmul(
                            out_ps[:],
                            agg_sc[h][c][:, ic * 128:(ic + 1) * 128],
                            u_bf[:, c, :],
                            start=c == 0,
                            stop=c == 3,
                        )
                    out_sb = osb_pool.tile([128, D], F32, tag="osb",
                                           name=f"osb{h}{ic}")
                    # out = relu(out_raw / deg): positive scale commutes with
                    # relu, applied per partition in the activation
                    nc.scalar.activation(
                        out_sb[:], out_ps[:],
                        mybir.ActivationFunctionType.Relu,
                        scale=recipt[h][:, ic:ic + 1],
                    )
                    nc.sync.dma_start(out_d[:, h * 4 + ic, :], out_sb[:])

            close_chunk(1, 0, "act")
            close_chunk(1, 1, "dve")
            deg_pipeline(1, "agg10", "agg11")
            close_chunk(1, 2, "act")
            close_chunk(1, 3, "dve")
            close_chunk(0, 0, "act")
            out_pipeline(1, "agg12", "agg13")
            close_chunk(0, 1, "dve")
            deg_pipeline(0, "agg00", "agg01")
            close_chunk(0, 2, "act")
            close_chunk(0, 3, "dve")
            out_pipeline(0, "agg02", "agg03")

    nc.compile()
    return nc


def _get_compiled():
    global _compiled
    if _compiled is None:
        _compiled = _build()
    return _compiled


def _run(x, adj, u, **spmd_kwargs):
    nc = _get_compiled()
    x = np.asarray(x, dtype=np.float32)
    adj = np.asarray(adj, dtype=np.int32)
    u = np.asarray(u, dtype=np.float32)

    # x[t*128+p, d] -> x_r[p, t, d]
    x_r = np.ascontiguousarray(x.reshape(NJ, 128, D).transpose(1, 0, 2))
    # U^T[c*128+p, k] -> ut_r[p, c, k]
    ut_r = np.ascontiguousarray(u.T.reshape(4, 128, D).transpose(1, 0, 2))
    in_maps = []
    for core in range(NCORES):
        shard = adj[:, core * SH:(core + 1) * SH]
        # shard[t*128+p, h*512+d] -> adj_r[h, p, t, d]
        adj_r = np.ascontiguousarray(
            shard.reshape(NJ, 128, 2, D).transpose(2, 1, 0, 3)
        )
        in_maps.append({"x": x_r, "ut": ut_r, "adj": adj_r})

    res = run_bass_kernel_spmd(nc, in_maps, core_ids=list(range(NCORES)), **spmd_kwargs)
    # out_r[p, hic, k] -> out[hic*128+p, k], then stack core slabs
    out = np.concatenate(
        [
            res.results[c]["out"].transpose(1, 0, 2).reshape(SH, D)
            for c in range(NCORES)
        ],
        axis=0,
    )
    return out, res


def kernel(x, adj, U):
    out, _ = _run(x, adj, U)
    return out


# revision 8
# speedup vs baseline: 1.0483x; 1.0483x over previous
"""GNN message-passing layer on 8 TRN2 NeuronCores.

Computes out = relu((adj^T @ x / deg) @ U^T) for N=8192 nodes, D=512 dims.

Sharding: columns of adj (= output rows) are split across the 8 cores;
x and U are replicated, so each core computes a [1024, 512] output slab
with no collectives.

Host-side restaging (pure layout shuffles, no arithmetic): every DRAM
tensor is laid out partition-major so each SBUF partition reads one long
contiguous run; the int32->bf16 cast rides the SWDGE DMA engines.

The per-core DMA bus (~360-385 GB/s) is the binding roofline (53.3 MB
of traffic), so the kernel is one fused j-loop: both 512-column output
halves accumulate simultaneously in all 8 PSUM banks while adj (both
halves) and x stream together - the DMA queues stay saturated for the
whole kernel.

deg is computed entirely off the PE: bf16 per-partition partials on the
DVE (exact: counts <= 64), lane-folded in place, partition-summed by a
gpsimd partition_all_reduce (replicated f32 output), reciprocal on the
DVE; the divide then fuses into PSUM evacuation as an elementwise DVE
multiply, so no PE transposes and no PSUM bank is ever needed for deg.

Ramp/tail shaping: the first and last groups are split into sub-DMAs so
the PE starts ~3us in and the last column-half (h0) finishes only ~2us
after its final adj byte; h1's adj is streamed entirely before h0's
final group, letting h1's deg/evac/output pipeline hide inside h0's
tail streaming window.
"""

import sys

if "/opt/trn_rl_repo" not in sys.path:
    sys.path.insert(0, "/opt/trn_rl_repo")

import numpy as np

import concourse.bacc as bacc
import concourse.bass_isa as bass_isa
import concourse.mybir as mybir
import concourse.tile as tile
from concourse.bass_utils import run_bass_kernel_spmd

N = 8192          # nodes
D = 512           # node dim
NCORES = 8
SH = N // NCORES  # 1024 adj columns (output rows) per core
NJ = N // 128     # 64 contraction tiles
XG = 8            # j-tiles per load group
NG = NJ // XG     # 8 groups
F32 = mybir.dt.float32
BF16 = mybir.dt.bfloat16
I32 = mybir.dt.int32

_compiled = None


def _build():
    nc = bacc.Bacc("TRN2", target_bir_lowering=False, debug=False, num_devices=NCORES)
    # partition-major layouts (see _run for the host-side shuffles)
    x_d = nc.dram_tensor("x", [128, NJ, D], F32, kind="ExternalInput").ap()
    adj_d = nc.dram_tensor("adj", [2, 128, NJ, D], I32, kind="ExternalInput").ap()
    ut_d = nc.dram_tensor("ut", [128, 4, D], F32, kind="ExternalInput").ap()
    out_d = nc.dram_tensor("out", [128, 8, D], F32, kind="ExternalOutput").ap()

    LAST = NG - 1

    with tile.TileContext(nc) as tc:
        with (
            tc.tile_pool(name="xw", bufs=5) as xw_pool,
            tc.tile_pool(name="xs", bufs=2) as xs_pool,
            tc.tile_pool(name="abf", bufs=6) as abf_pool,
            tc.tile_pool(name="cons", bufs=1) as cons_pool,
            tc.tile_pool(name="degp", bufs=1) as degp_pool,
            tc.tile_pool(name="evac", bufs=1) as evac_pool,
            tc.tile_pool(name="osb", bufs=2) as osb_pool,
            tc.tile_pool(name="pacc", bufs=1, space="PSUM") as pacc_pool,
        ):
            ones = cons_pool.tile([128, 128], BF16)
            nc.vector.memset(ones[:], 1.0)

            # 8 PSUM accumulators: [h][c] chunk of aggT
            agg_ps = [
                [
                    pacc_pool.tile([128, D], F32, tag=f"agg{h}{c}", name=f"agg{h}{c}")
                    for c in range(4)
                ]
                for h in range(2)
            ]
            agg_sc = [
                [
                    evac_pool.tile([128, D], BF16, tag=f"aggsc{h}{c}",
                                   name=f"aggsc{h}{c}")
                    for c in range(4)
                ]
                for h in range(2)
            ]
            # per-partition partial degree counts; lane values stay small so
            # bf16 accumulation is exact.  degp holds groups 0..NG-2 (folded
            # in place before the last group); degf holds the last group.
            degp = [
                degp_pool.tile([128, XG, D], BF16, tag=f"degp{h}", name=f"degp{h}")
                for h in range(2)
            ]
            degf = [
                degp_pool.tile([128, 2, D], BF16, tag=f"degf{h}", name=f"degf{h}")
                for h in range(2)
            ]
            for h in range(2):
                nc.vector.memset(degp[h][:], 0.0)
                nc.vector.memset(degf[h][:], 0.0)

            # startup PE filler: ramp the clock / pstate while the first
            # sub-group's DMA lands; garbage lands in agg banks and is reset
            # by the j==0 start=True matmuls
            for f in range(6):
                nc.tensor.matmul(
                    agg_ps[f % 2][(f // 2) % 4][:, 0:128], ones[:], ones[:],
                    start=True, stop=True, skip_group_check=True,
                )

            xg_tiles = [None] * NG
            a_tiles = [[None] * NG for _ in range(2)]

            def load_x(g, subs=1):
                xs = xs_pool.tile([128, XG, D], F32, tag="xs", name=f"xs{g}")
                xg = xw_pool.tile([128, XG, D], BF16, tag="xg", name=f"xg{g}")
                step = XG // subs
                for s in range(subs):
                    lo, hi = s * step, (s + 1) * step
                    nc.sync.dma_start(
                        xs[:, lo:hi, :], x_d[:, g * XG + lo:g * XG + hi, :]
                    )
                    nc.scalar.copy(xg[:, lo:hi, :], xs[:, lo:hi, :])
                xg_tiles[g] = xg

            def load_adj(h, g, subs=1):
                a_bf = abf_pool.tile([128, XG, D], BF16, tag="abf",
                                     name=f"abf{h}_{g}")
                step = XG // subs
                for s in range(subs):
                    lo, hi = s * step, (s + 1) * step
                    nc.gpsimd.dma_start(
                        a_bf[:, lo:hi, :],
                        adj_d[h, :, g * XG + lo:g * XG + hi, :],
                    )
                a_tiles[h][g] = a_bf

            # prime the pipeline: first group fine-grained so the PE starts
            # early; x runs one group further ahead than adj so the tail x
            # cast never lands on the critical path
            load_x(0, subs=4)
            load_adj(1, 0, subs=2)
            load_adj(0, 0, subs=2)
            load_x(1)
            load_adj(1, 1)
            load_adj(0, 1)
            load_x(2)

            def mm_tile(h, g, t, c):
                j = g * XG + t
                nc.tensor.matmul(
                    agg_ps[h][c][:],
                    xg_tiles[g][:, t, c * 128:(c + 1) * 128],
                    a_tiles[h][g][:, t, :],
                    start=j == 0,
                    stop=j == NJ - 1,
                )

            # ---- steady-state j-loop over groups 0..NG-2 ----
            for g in range(NG - 1):
                if g + 3 < NG:
                    load_x(g + 3)
                if g + 2 < NG:
                    if g + 2 == LAST:
                        # last group: h1 entirely before h0, 2-tile sub-DMAs
                        load_adj(1, LAST, subs=4)
                        load_adj(0, LAST, subs=4)
                    else:
                        load_adj(1, g + 2, subs=1)
                        load_adj(0, g + 2, subs=1)
                if g == 4:
                    # U rides the idle HWDGE queue late, cast on ACT
                    u_s = cons_pool.tile([128, 4, D], F32, tag="us", name="us")
                    nc.sync.dma_start(u_s[:], ut_d[:])
                    u_bf = cons_pool.tile([128, 4, D], BF16, tag="ubf", name="ubf")
                    nc.scalar.copy(u_bf[:], u_s[:])
                for h in range(2):
                    nc.vector.tensor_add(degp[h][:], degp[h][:], a_tiles[h][g][:])
                if g == 0:
                    # group 0: h1's data lands fully before h0's
                    for h in (1, 0):
                        for t in range(XG):
                            for c in range(4):
                                mm_tile(h, g, t, c)
                else:
                    for t in range(XG):
                        for c in range(4):
                            mm_tile(1, g, t, c)
                            mm_tile(0, g, t, c)

            # in-place lane fold of degp (groups 0..NG-2): 8 -> 1 lanes
            for h in range(2):
                nc.vector.tensor_add(
                    degp[h][:, 0:4, :], degp[h][:, 0:4, :], degp[h][:, 4:8, :]
                )
                nc.vector.tensor_add(
                    degp[h][:, 0:2, :], degp[h][:, 0:2, :], degp[h][:, 2:4, :]
                )
                nc.vector.tensor_add(
                    degp[h][:, 0, :], degp[h][:, 0, :], degp[h][:, 1, :]
                )

            degsum = [None, None]
            recip = [None, None]

            def deg_finish(h):
                # last group's 8 lanes -> degf (2 lanes), + folded degp
                a = a_tiles[h][LAST]
                for s in range(4):
                    for tt in range(2):
                        nc.vector.tensor_add(
                            degf[h][:, tt, :], degf[h][:, tt, :],
                            a[:, 2 * s + tt, :],
                        )
                ds = evac_pool.tile([128, D], F32, tag=f"degsum{h}",
                                    name=f"degsum{h}")
                nc.vector.tensor_add(degf[h][:, 0, :], degf[h][:, 0, :],
                                     degf[h][:, 1, :])
                nc.vector.tensor_add(ds[:], degf[h][:, 0, :], degp[h][:, 0, :])
                degsum[h] = ds
                # partition sum, replicated f32 across all partitions
                dt = evac_pool.tile([128, D], F32, tag=f"degtot{h}",
                                    name=f"degtot{h}")
                nc.gpsimd.partition_all_reduce(
                    dt[:], ds[:], channels=128, reduce_op=bass_isa.ReduceOp.add
                )
                rc = evac_pool.tile([128, D], F32, tag=f"recip{h}",
                                    name=f"recip{h}")
                nc.vector.reciprocal_approx_fast(rc[:], dt[:])
                recip[h] = rc

            def evac_divide(h, c):
                # agg_sc = agg_ps / deg, fused into the PSUM evacuation
                nc.vector.tensor_mul(agg_sc[h][c][:], agg_ps[h][c][:],
                                     recip[h][:])

            def out_pipeline(h, bank0, bank1, ics):
                for ic in ics:
                    out_ps = pacc_pool.tile([128, D], F32,
                                            tag=(bank0 if ic % 2 == 0 else bank1),
                                            name=f"outps{h}{ic}")
                    for c in range(4):
                        nc.tensor.matmul(
                            out_ps[:],
                            agg_sc[h][c][:, ic * 128:(ic + 1) * 128],
                            u_bf[:, c, :],
                            start=c == 0,
                            stop=c == 3,
                        )
                    out_sb = osb_pool.tile([128, D], F32, tag="osb",
                                           name=f"osb{h}{ic}")
                    nc.scalar.activation(
                        out_sb[:], out_ps[:],
                        mybir.ActivationFunctionType.Relu,
                    )
                    nc.sync.dma_start(out_d[:, h * 4 + ic, :], out_sb[:])

            # ---- final group ----
            g = LAST
            # h1 matmuls sub-major (data arrives per 2-tile sub)
            for s in range(4):
                for tt in range(2):
                    for c in range(4):
                        mm_tile(1, g, 2 * s + tt, c)
            deg_finish(1)
            for c in range(4):
                evac_divide(1, c)
            # h0 subs stream while h1's deg/evac/out pipeline runs
            for s in range(4):
                for tt in range(2):
                    for c in range(4):
                        mm_tile(0, g, 2 * s + tt, c)
                if s == 0:
                    deg_finish(0)
                if s == 1:
                    out_pipeline(1, "agg10", "agg11", (0, 1))
                if s == 2:
                    out_pipeline(1, "agg12", "agg13", (2, 3))
            for c in range(4):
                evac_divide(0, c)
            out_pipeline(0, "agg00", "agg01", (0, 1, 2, 3))

    nc.compile()
    return nc


def _get_compiled():
    global _compiled
    if _compiled is None:
        _compiled = _build()
    return _compiled


def _run(x, adj, u, **spmd_kwargs):
    nc = _get_compiled()
    x = np.asarray(x, dtype=np.float32)
    adj = np.asarray(adj, dtype=np.int32)
    u = np.asarray(u, dtype=np.float32)

    # x[t*128+p, d] -> x_r[p, t, d]
    x_r = np.ascontiguousarray(x.reshape(NJ, 128, D).transpose(1, 0, 2))
    # U^T[c*128+p, k] -> ut_r[p, c, k]
    ut_r = np.ascontiguousarray(u.T.reshape(4, 128, D).transpose(1, 0, 2))
    in_maps = []
    for core in range(NCORES):
        shard = adj[:, core * SH:(core + 1) * SH]
        # shard[t*128+p, h*512+d] -> adj_r[h, p, t, d]
        adj_r = np.ascontiguousarray(
            shard.reshape(NJ, 128, 2, D).transpose(2, 1, 0, 3)
        )
        in_maps.append({"x": x_r, "ut": ut_r, "adj": adj_r})

    res = run_bass_kernel_spmd(nc, in_maps, core_ids=list(range(NCORES)), **spmd_kwargs)
    # out_r[p, hic, k] -> out[hic*128+p, k], then stack core slabs
    out = np.concatenate(
        [
            res.results[c]["out"].transpose(1, 0, 2).reshape(SH, D)
            for c in range(NCORES)
        ],
        axis=0,
    )
    return out, res


def kernel(x, adj, U):
    out, _ = _run(x, adj, U)
    return out


# revision 11
# speedup vs baseline: 1.1751x; 1.1210x over previous
"""GNN message-passing layer on 8 TRN2 NeuronCores.

Computes out = relu((adj^T @ x / deg) @ U^T) for N=8192 nodes, D=512 dims.

Sharding: columns of adj (= output rows) are split across the 8 cores;
x and U are replicated, so each core computes a [1024, 512] output slab
with no collectives.

Host-side restaging (pure layout shuffles, no arithmetic): every DRAM
tensor is laid out partition-major so each SBUF partition reads one long
contiguous run; the int32->bf16 cast rides the SWDGE DMA engines.

The per-core DMA bus (~360-385 GB/s) is the binding roofline (53.3 MB
of traffic), so the kernel is one fused j-loop: both 512-column output
halves accumulate simultaneously in all 8 PSUM banks while adj (both
halves) and x stream together - the DMA queues stay saturated for the
whole kernel.

deg is computed entirely off the PE: bf16 per-partition partials on the
DVE (exact: counts <= 64), lane-folded in place, partition-summed by a
gpsimd partition_all_reduce (replicated f32 output), reciprocal on the
DVE; the divide then fuses into PSUM evacuation as an elementwise DVE
multiply, so no PE transposes and no PSUM bank is ever needed for deg.

Ramp/tail shaping: the first and last groups are split into sub-DMAs so
the PE starts ~3us in and the last column-half (h0) finishes only ~2us
after its final adj byte; h1's adj is streamed entirely before h0's
final group, letting h1's deg/evac/output pipeline hide inside h0's
tail streaming window.
"""

import sys

if "/opt/trn_rl_repo" not in sys.path:
    sys.path.insert(0, "/opt/trn_rl_repo")

import numpy as np

import concourse.bacc as bacc
import concourse.bass_isa as bass_isa
import concourse.mybir as mybir
import concourse.tile as tile
from concourse.bass_utils import run_bass_kernel_spmd

N = 8192          # nodes
D = 512           # node dim
NCORES = 8
SH = N // NCORES  # 1024 adj columns (output rows) per core
NJ = N // 128     # 64 contraction tiles
XG = 8            # j-tiles per load group
NG = NJ // XG     # 8 groups
F32 = mybir.dt.float32
BF16 = mybir.dt.bfloat16
I32 = mybir.dt.int32

_compiled = None


def _build():
    nc = bacc.Bacc("TRN2", target_bir_lowering=False, debug=False, num_devices=NCORES)
    # partition-major layouts (see _run for the host-side shuffles)
    x_d = nc.dram_tensor("x", [128, NJ, D], F32, kind="ExternalInput").ap()
    adj_d = nc.dram_tensor("adj", [2, 128, NJ, D], I32, kind="ExternalInput").ap()
    ut_d = nc.dram_tensor("ut", [128, 4, D], F32, kind="ExternalInput").ap()
    out_d = nc.dram_tensor("out", [128, 8, D], F32, kind="ExternalOutput").ap()

    LAST = NG - 1

    with tile.TileContext(nc) as tc:
        with (
            tc.tile_pool(name="xw", bufs=5) as xw_pool,
            tc.tile_pool(name="xs", bufs=3) as xs_pool,
            tc.tile_pool(name="abf", bufs=6) as abf_pool,
            tc.tile_pool(name="cons", bufs=1) as cons_pool,
            tc.tile_pool(name="degp", bufs=1) as degp_pool,
            tc.tile_pool(name="evac", bufs=1) as evac_pool,
            tc.tile_pool(name="osb", bufs=2) as osb_pool,
            tc.tile_pool(name="pacc", bufs=1, space="PSUM") as pacc_pool,
        ):
            ones = cons_pool.tile([128, 128], BF16)
            nc.vector.memset(ones[:], 1.0)

            # 8 PSUM accumulators: [h][c] chunk of aggT
            agg_ps = [
                [
                    pacc_pool.tile([128, D], F32, tag=f"agg{h}{c}", name=f"agg{h}{c}")
                    for c in range(4)
                ]
                for h in range(2)
            ]
            agg_sc = [
                [
                    evac_pool.tile([128, D], BF16, tag=f"aggsc{h}{c}",
                                   name=f"aggsc{h}{c}")
                    for c in range(4)
                ]
                for h in range(2)
            ]
            # per-partition partial degree counts; lane values stay small so
            # bf16 accumulation is exact.  degp holds groups 0..NG-2 (folded
            # in place before the last group); degf holds the last group.
            degp = [
                degp_pool.tile([128, XG, D], BF16, tag=f"degp{h}", name=f"degp{h}")
                for h in range(2)
            ]
            degf = [
                degp_pool.tile([128, 2, D], BF16, tag=f"degf{h}", name=f"degf{h}")
                for h in range(2)
            ]
            for h in range(2):
                nc.vector.memset(degp[h][:], 0.0)
                nc.vector.memset(degf[h][:], 0.0)

            # startup PE filler: ramp the clock / pstate while the first
            # sub-group's DMA lands; garbage lands in agg banks and is reset
            # by the j==0 start=True matmuls
            for f in range(6):
                nc.tensor.matmul(
                    agg_ps[f % 2][(f // 2) % 4][:, 0:128], ones[:], ones[:],
                    start=True, stop=True, skip_group_check=True,
                )

            xg_tiles = [None] * NG
            a_tiles = [[None] * NG for _ in range(2)]

            def load_x(g, subs=1):
                xs = xs_pool.tile([128, XG, D], F32, tag="xs", name=f"xs{g}")
                xg = xw_pool.tile([128, XG, D], BF16, tag="xg", name=f"xg{g}")
                step = XG // subs
                for s in range(subs):
                    lo, hi = s * step, (s + 1) * step
                    nc.sync.dma_start(
                        xs[:, lo:hi, :], x_d[:, g * XG + lo:g * XG + hi, :]
                    )
                    nc.scalar.copy(xg[:, lo:hi, :], xs[:, lo:hi, :])
                xg_tiles[g] = xg

            def load_adj(h, g, subs=1):
                a_bf = abf_pool.tile([128, XG, D], BF16, tag="abf",
                                     name=f"abf{h}_{g}")
                step = XG // subs
                for s in range(subs):
                    lo, hi = s * step, (s + 1) * step
                    nc.gpsimd.dma_start(
                        a_bf[:, lo:hi, :],
                        adj_d[h, :, g * XG + lo:g * XG + hi, :],
                    )
                a_tiles[h][g] = a_bf

            # prime the pipeline: first group fine-grained so the PE starts
            # early; x runs several groups ahead of adj so the cast chain
            # (sync DMA -> ACT cast) never gates the matmul stream
            load_x(0, subs=4)
            load_adj(1, 0, subs=2)
            load_adj(0, 0, subs=2)
            load_x(1, subs=2)
            load_adj(1, 1)
            load_adj(0, 1)
            load_x(2, subs=2)
            load_x(3, subs=2)

            def mm_tile(h, g, t, c):
                j = g * XG + t
                nc.tensor.matmul(
                    agg_ps[h][c][:],
                    xg_tiles[g][:, t, c * 128:(c + 1) * 128],
                    a_tiles[h][g][:, t, :],
                    start=j == 0,
                    stop=j == NJ - 1,
                )

            # ---- steady-state j-loop over groups 0..NG-2 ----
            for g in range(NG - 1):
                if g + 4 < NG:
                    load_x(g + 4, subs=2)
                if g + 2 < NG:
                    if g + 2 == LAST:
                        # last group: h1 entirely before h0, 2-tile sub-DMAs
                        load_adj(1, LAST, subs=4)
                        load_adj(0, LAST, subs=4)
                    else:
                        load_adj(1, g + 2, subs=1)
                        load_adj(0, g + 2, subs=1)
                if g == 4:
                    # U rides the idle HWDGE queue late, cast on ACT
                    u_s = cons_pool.tile([128, 4, D], F32, tag="us", name="us")
                    nc.sync.dma_start(u_s[:], ut_d[:])
                    u_bf = cons_pool.tile([128, 4, D], BF16, tag="ubf", name="ubf")
                    nc.scalar.copy(u_bf[:], u_s[:])
                for h in range(2):
                    nc.vector.tensor_add(degp[h][:], degp[h][:], a_tiles[h][g][:])
                # h1's tile lands fully before h0's (SWDGE queue order), so
                # running all h1 matmuls first hides h0's remaining arrival
                for h in (1, 0):
                    for t in range(XG):
                        for c in range(4):
                            mm_tile(h, g, t, c)

            # in-place lane fold of degp (groups 0..NG-2): 8 -> 1 lanes
            for h in range(2):
                nc.vector.tensor_add(
                    degp[h][:, 0:4, :], degp[h][:, 0:4, :], degp[h][:, 4:8, :]
                )
                nc.vector.tensor_add(
                    degp[h][:, 0:2, :], degp[h][:, 0:2, :], degp[h][:, 2:4, :]
                )
                nc.vector.tensor_add(
                    degp[h][:, 0, :], degp[h][:, 0, :], degp[h][:, 1, :]
                )

            degsum = [None, None]
            recip = [None, None]

            def deg_finish(h):
                # last group's 8 lanes -> degf (2 lanes), + folded degp
                a = a_tiles[h][LAST]
                for s in range(4):
                    for tt in range(2):
                        nc.vector.tensor_add(
                            degf[h][:, tt, :], degf[h][:, tt, :],
                            a[:, 2 * s + tt, :],
                        )
                ds = evac_pool.tile([128, D], F32, tag=f"degsum{h}",
                                    name=f"degsum{h}")
                nc.vector.tensor_add(degf[h][:, 0, :], degf[h][:, 0, :],
                                     degf[h][:, 1, :])
                nc.vector.tensor_add(ds[:], degf[h][:, 0, :], degp[h][:, 0, :])
                degsum[h] = ds
                # partition sum, replicated f32 across all partitions
                dt = evac_pool.tile([128, D], F32, tag=f"degtot{h}",
                                    name=f"degtot{h}")
                nc.gpsimd.partition_all_reduce(
                    dt[:], ds[:], channels=128, reduce_op=bass_isa.ReduceOp.add
                )
                rc = evac_pool.tile([128, D], F32, tag=f"recip{h}",
                                    name=f"recip{h}")
                nc.vector.reciprocal_approx_fast(rc[:], dt[:])
                recip[h] = rc

            def evac_divide(h, c):
                # agg_sc = agg_ps / deg, fused into the PSUM evacuation
                nc.vector.tensor_mul(agg_sc[h][c][:], agg_ps[h][c][:],
                                     recip[h][:])

            def out_pipeline(h, bank0, bank1, ics):
                for ic in ics:
                    out_ps = pacc_pool.tile([128, D], F32,
                                            tag=(bank0 if ic % 2 == 0 else bank1),
                                            name=f"outps{h}{ic}")
                    for c in range(4):
                        nc.tensor.matmul(
                            out_ps[:],
                            agg_sc[h][c][:, ic * 128:(ic + 1) * 128],
                            u_bf[:, c, :],
                            start=c == 0,
                            stop=c == 3,
                        )
                    out_sb = osb_pool.tile([128, D], F32, tag="osb",
                                           name=f"osb{h}{ic}")
                    nc.scalar.activation(
                        out_sb[:], out_ps[:],
                        mybir.ActivationFunctionType.Relu,
                    )
                    nc.sync.dma_start(out_d[:, h * 4 + ic, :], out_sb[:])

            # ---- final group ----
            g = LAST
            # h1 matmuls sub-major (data arrives per 2-tile sub)
            for s in range(4):
                for tt in range(2):
                    for c in range(4):
                        mm_tile(1, g, 2 * s + tt, c)
            deg_finish(1)
            for c in range(4):
                evac_divide(1, c)
            # h0 subs stream while h1's deg/evac/out pipeline runs
            for s in range(4):
                for tt in range(2):
                    for c in range(4):
                        mm_tile(0, g, 2 * s + tt, c)
                if s == 0:
                    deg_finish(0)
                if s == 1:
                    out_pipeline(1, "agg10", "agg11", (0, 1))
                if s == 2:
                    out_pipeline(1, "agg12", "agg13", (2, 3))
            for c in range(4):
                evac_divide(0, c)
            out_pipeline(0, "agg00", "agg01", (0, 1, 2, 3))

    nc.compile()
    return nc


def _get_compiled():
    global _compiled
    if _compiled is None:
        _compiled = _build()
    return _compiled


def _run(x, adj, u, **spmd_kwargs):
    nc = _get_compiled()
    x = np.asarray(x, dtype=np.float32)
    adj = np.asarray(adj, dtype=np.int32)
    u = np.asarray(u, dtype=np.float32)

    # x[t*128+p, d] -> x_r[p, t, d]
    x_r = np.ascontiguousarray(x.reshape(NJ, 128, D).transpose(1, 0, 2))
    # U^T[c*128+p, k] -> ut_r[p, c, k]
    ut_r = np.ascontiguousarray(u.T.reshape(4, 128, D).transpose(1, 0, 2))
    in_maps = []
    for core in range(NCORES):
        shard = adj[:, core * SH:(core + 1) * SH]
        # shard[t*128+p, h*512+d] -> adj_r[h, p, t, d]
        adj_r = np.ascontiguousarray(
            shard.reshape(NJ, 128, 2, D).transpose(2, 1, 0, 3)
        )
        in_maps.append({"x": x_r, "ut": ut_r, "adj": adj_r})

    res = run_bass_kernel_spmd(nc, in_maps, core_ids=list(range(NCORES)), **spmd_kwargs)
    # out_r[p, hic, k] -> out[hic*128+p, k], then stack core slabs
    out = np.concatenate(
        [
            res.results[c]["out"].transpose(1, 0, 2).reshape(SH, D)
            for c in range(NCORES)
        ],
        axis=0,
    )
    return out, res


def kernel(x, adj, U):
    out, _ = _run(x, adj, U)
    return out
